# revision 22
# baseline (speedup 1.0000x reference)
"""AdaLN DiT block on 8 Trainium2 NeuronCores.

Sharding: core c owns tokens [h*512,(h+1)*512) of batch b, where b=c//2,
h=c%2. Attention is sharded-Q: each core computes q/k/v for its own 512
tokens with the full projection weights, pair-AllGathers kT and v so it
has the full-sequence keys/values of its batch, then runs all 16 heads
for its own 512 query tokens. Wo and the whole MLP are token-local with
full (bf16, host-pre-transposed) weights streamed from HBM. The adaLN
modulation vector is computed per-core for its own batch. No reduce
collectives; the only comm is the per-pair kT/v AllGather.

Everything on-chip is feature-major ([D-on-partitions, tokens]); LN
statistics are computed with ones-vector matmuls on the TensorEngine
(partition-dim reduction), so no layout transposes are needed anywhere.
Matmuls run in bf16 (fp32 PSUM accumulation); the residual stream stays
fp32.

PSUM budget (8 banks): tag "big" [128,1024] x2 = 4 banks (scores, fc1),
tag "proj" [128,512] x2 = 2 (projections, LN stats/bcasts), tag "av"
[128,512] x2 = 2 (adaLN accum early, attention AV / denom-bcast later).
NOTE: matmul start=True clears the WHOLE psum bank, so only the first
matmul touching a bank may set it; later column-groups rely on the
per-element has_written bits for first-touch overwrite.
"""

import numpy as np

B, S, D, H, HID = 4, 1024, 1024, 16, 4096
DK = D // H  # 64
N_CORES = 8
TOK = 512
HT = TOK // 2
EPS = 1e-6
KT = 8    # 128-row blocks in D
HC = 32   # 128-row blocks in HID

_cached = {}
DEBUG = False


def _build():
    import contextlib
    import concourse.bass as bass  # noqa: F401
    import concourse.tile as tile
    from concourse import bacc, mybir

    f32 = mybir.dt.float32
    bf16 = mybir.dt.bfloat16
    AF = mybir.ActivationFunctionType
    OP = mybir.AluOpType

    nc = bacc.Bacc("TRN2", target_bir_lowering=False, debug=False,
                   num_devices=N_CORES)

    # ---- per-core external I/O ----
    x_feat = nc.dram_tensor("x_feat", [D, TOK], f32, kind="ExternalInput")
    condT = nc.dram_tensor("condT", [128, 8], f32, kind="ExternalInput")
    wq_t = nc.dram_tensor("wq_t", [KT, 128, D], bf16, kind="ExternalInput")
    wk_t = nc.dram_tensor("wk_t", [KT, 128, D], bf16, kind="ExternalInput")
    wo_t = nc.dram_tensor("wo_t", [KT, 128, D], bf16, kind="ExternalInput")
    wvT = nc.dram_tensor("wvT", [D, D], bf16, kind="ExternalInput")
    w1_t = nc.dram_tensor("w1_t", [KT, 128, 8 * 512], bf16, kind="ExternalInput")
    w2_t = nc.dram_tensor("w2_t", [KT, 2, 128, 2048], bf16, kind="ExternalInput")
    wadaT = nc.dram_tensor("wadaT", [D, 6 * D], bf16, kind="ExternalInput")
    # packed per-partition bias columns (fp32): 0..47 bada, 48..55 bq,
    # 56..63 bk, 64..71 bo, 72..79 b2
    biasc = nc.dram_tensor("biasc", [128, 80], f32, kind="ExternalInput")
    bv_row = nc.dram_tensor("bv_row", [1, D], bf16, kind="ExternalInput")
    b1_row = nc.dram_tensor("b1_row", [1, HID], bf16, kind="ExternalInput")
    onehot_d = nc.dram_tensor("onehot_d", [16, 1024], bf16, kind="ExternalInput")
    out_feat = nc.dram_tensor("out_feat", [D, TOK], f32, kind="ExternalOutput")
    if DEBUG:
        dbg_mod = nc.dram_tensor("dbg_mod", [128, 48], f32, kind="ExternalOutput")
        dbg_h1T = nc.dram_tensor("dbg_h1T", [D, TOK], f32, kind="ExternalOutput")
        dbg_qT = nc.dram_tensor("dbg_qT", [D, TOK], f32, kind="ExternalOutput")
        dbg_kago = nc.dram_tensor("dbg_kago", [2 * D, TOK], f32, kind="ExternalOutput")
        dbg_vago = nc.dram_tensor("dbg_vago", [2 * TOK, D], f32, kind="ExternalOutput")
        dbg_attnT = nc.dram_tensor("dbg_attnT", [D, TOK], f32, kind="ExternalOutput")
        dbg_x2T = nc.dram_tensor("dbg_x2T", [D, TOK], f32, kind="ExternalOutput")
        dbg_h2T = nc.dram_tensor("dbg_h2T", [D, TOK], f32, kind="ExternalOutput")

    PAIRS = [[2 * i, 2 * i + 1] for i in range(B)]

    with tile.TileContext(nc) as tc:
        ctx = contextlib.ExitStack()
        consts = ctx.enter_context(tc.tile_pool(name="consts", bufs=1))
        persist = ctx.enter_context(tc.tile_pool(name="persist", bufs=1))
        wpool = ctx.enter_context(tc.tile_pool(name="wpool", bufs=3))
        trans = ctx.enter_context(tc.tile_pool(name="trans", bufs=3))
        pT_pool = ctx.enter_context(tc.tile_pool(name="pTp", bufs=2))
        kv_pool = ctx.enter_context(tc.tile_pool(name="kvp", bufs=2))
        dram = ctx.enter_context(tc.tile_pool(name="dram", bufs=1, space="DRAM"))
        psum = ctx.enter_context(tc.tile_pool(name="psum", bufs=2, space="PSUM"))

        # ---- internal DRAM for the pair AllGather ----
        k_ag_in = dram.tile([D, TOK], bf16, tag="k_ag_in")
        k_ag_out = dram.tile([2 * D, TOK], bf16, tag="k_ag_out")
        v_ag_in = dram.tile([TOK, D], bf16, tag="v_ag_in")
        v_ag_out = dram.tile([2 * TOK, D], bf16, tag="v_ag_out")

        # ---------- constants ----------
        bias_sb = consts.tile([128, 80], f32)
        nc.gpsimd.dma_start(bias_sb[:], biasc[:])
        cond_sb = consts.tile([128, 8], f32)
        nc.gpsimd.dma_start(cond_sb[:], condT[:])
        bvr_sb = consts.tile([1, D], bf16)
        nc.gpsimd.dma_start(bvr_sb[:], bv_row[:])
        b1r_sb = consts.tile([1, HID], bf16)
        nc.gpsimd.dma_start(b1r_sb[:], b1_row[:])
        onehot = consts.tile([16, 1024], bf16)
        nc.gpsimd.dma_start(onehot[:], onehot_d[:])
        eps_sb = consts.tile([1, 1], f32)
        nc.vector.memset(eps_sb[:], EPS)
        ones_m = consts.tile([1, 128], bf16)
        nc.vector.memset(ones_m[:], 1.0)
        ones_col = consts.tile([128, 1], bf16)
        nc.vector.memset(ones_col[:], 1.0)
        ones_tok = consts.tile([1, HT], bf16)
        nc.vector.memset(ones_tok[:], 1.0)

        def bcol(i):
            return bias_sb[:, i:i + 1]

        def dump(dst_slice, src_tile):
            if DEBUG:
                tmp = trans.tile(list(src_tile.shape), f32, tag="dbg",
                                 name="dbgtmp", bufs=2)
                nc.vector.tensor_copy(out=tmp[:], in_=src_tile[:])
                nc.sync.dma_start(dst_slice, tmp[:])

        # ---------- adaLN modulation (own batch) ----------
        silu_sb = consts.tile([128, 8], bf16)
        nc.scalar.activation(silu_sb[:], cond_sb[:], AF.Silu)

        mod_sb = consts.tile([128, 48], f32)
        mod1p_sb = consts.tile([128, 48], f32)

        def mod_part(ps, col0, ncols, bank_first=True):
            for k in range(KT):
                wt = wpool.tile([128, ncols * 128], bf16, tag="wadas", bufs=2)
                nc.sync.dma_start(
                    wt[:], wadaT[k * 128:(k + 1) * 128,
                                 col0 * 128:(col0 + ncols) * 128])
                for j in range(ncols):
                    nc.tensor.matmul(
                        ps[:, j:j + 1], lhsT=wt[:, j * 128:(j + 1) * 128],
                        rhs=silu_sb[:, k:k + 1],
                        start=(bank_first and k == 0 and j == 0),
                        stop=(k == KT - 1), skip_group_check=True)
            return ps

        # urgent: gama1 (blocks 0..7), beta1 (8..15)
        ps_mod = psum.tile([128, 16], f32, tag="av", bufs=2)
        mod_part(ps_mod[:, 0:8], 0, 8, bank_first=True)
        mod_part(ps_mod[:, 8:16], 8, 8, bank_first=False)
        nc.vector.tensor_tensor(out=mod_sb[:, 0:16], in0=ps_mod[:],
                                in1=bias_sb[:, 0:16], op=OP.add)
        nc.vector.tensor_scalar_add(mod1p_sb[:, 0:16], mod_sb[:, 0:16], 1.0)

        # ---------- feature-major LayerNorm via PE statistics ----------
        xfeat_sb = []
        for d in range(KT):
            xf = persist.tile([128, TOK], f32, tag=f"xfeat{d}", name=f"xf{d}")
            nc.gpsimd.dma_start(xf[:], x_feat[d * 128:(d + 1) * 128, :])
            xfeat_sb.append(xf)

        hT_tiles = {}

        def layer_norm_mod_feat(src_tiles, beta_blk, gama_blk, key):
            """src: 8 fp32 [128, TOK] feature-major tiles ->
            hT = LN(src)*(1+beta)+gama, bf16 feature-major (persist)."""
            ps_sum = psum.tile([1, TOK], f32, tag="proj", name="ps_sum")
            ps_sq = psum.tile([1, TOK], f32, tag="proj", name="ps_sq")
            for d in range(KT):
                cb = trans.tile([128, TOK], bf16, tag="lncast", bufs=3,
                                name="lncast")
                nc.vector.tensor_copy(out=cb[:], in_=src_tiles[d][:])
                nc.tensor.matmul(ps_sum[:], lhsT=ones_col[:], rhs=cb[:],
                                 start=(d == 0), stop=(d == KT - 1))
                sq = trans.tile([128, TOK], bf16, tag="lnsq", bufs=3,
                                name="lnsq")
                nc.vector.tensor_tensor(out=sq[:], in0=cb[:],
                                        in1=cb[:], op=OP.mult)
                nc.tensor.matmul(ps_sq[:], lhsT=ones_col[:], rhs=sq[:],
                                 start=(d == 0), stop=(d == KT - 1))
            # stats in one [1, 5*TOK] fp32 tile: mu|ex2|var|std|rstd
            st = trans.tile([1, 5 * TOK], f32, tag="lnstat", name="lnstat",
                            bufs=2)
            mu, ex2 = st[:, 0:TOK], st[:, TOK:2 * TOK]
            var, std = st[:, 2 * TOK:3 * TOK], st[:, 3 * TOK:4 * TOK]
            rstd = st[:, 4 * TOK:5 * TOK]
            nc.vector.tensor_scalar(out=mu, in0=ps_sum[:], scalar1=1.0 / D,
                                    scalar2=None, op0=OP.mult)
            nc.vector.tensor_scalar(out=ex2, in0=ps_sq[:], scalar1=1.0 / D,
                                    scalar2=None, op0=OP.mult)
            nc.vector.tensor_tensor(out=var, in0=mu, in1=mu, op=OP.mult)
            nc.vector.tensor_tensor(out=var, in0=ex2, in1=var, op=OP.subtract)
            nc.scalar.activation(std, var, AF.Sqrt, bias=eps_sb[:], scale=1.0)
            nc.vector.reciprocal(rstd, std)
            stbf = trans.tile([1, 2 * TOK], bf16, tag="lnstatbf",
                              name="lnstatbf", bufs=2)
            mu_bf, rstd_bf = stbf[:, 0:TOK], stbf[:, TOK:2 * TOK]
            nc.vector.tensor_copy(out=mu_bf, in_=mu)
            nc.vector.tensor_copy(out=rstd_bf, in_=rstd)
            # broadcast mu and rstd across partitions via ones-matmul
            ps_mu = psum.tile([128, TOK], f32, tag="proj", name="ps_mu")
            nc.tensor.matmul(ps_mu[:], lhsT=ones_m[:], rhs=mu_bf,
                             start=True, stop=True)
            ps_rstd = psum.tile([128, TOK], f32, tag="proj", name="ps_rstd")
            nc.tensor.matmul(ps_rstd[:], lhsT=ones_m[:], rhs=rstd_bf,
                             start=True, stop=True)
            for d in range(KT):
                t1 = trans.tile([128, TOK], f32, tag="lnt", bufs=2,
                                name="lnt1")
                nc.vector.tensor_tensor(out=t1[:], in0=src_tiles[d][:],
                                        in1=ps_mu[:], op=OP.subtract)
                t2 = trans.tile([128, TOK], f32, tag="lnt", bufs=2,
                                name="lnt2")
                nc.vector.tensor_tensor(out=t2[:], in0=t1[:], in1=ps_rstd[:],
                                        op=OP.mult)
                ht = persist.tile([128, TOK], bf16, tag="hT", bufs=8,
                                  name="hT")
                nc.vector.tensor_scalar(
                    out=ht[:], in0=t2[:],
                    scalar1=mod1p_sb[:, beta_blk + d:beta_blk + d + 1],
                    scalar2=mod_sb[:, gama_blk + d:gama_blk + d + 1],
                    op0=OP.mult, op1=OP.add)
                hT_tiles[(key, d)] = ht
                if DEBUG:
                    dump((dbg_h1T if key == "h1" else dbg_h2T)
                         [d * 128:(d + 1) * 128, :], ht)

        layer_norm_mod_feat(xfeat_sb, 8, 0, "h1")
        h1T = [hT_tiles[("h1", d)] for d in range(KT)]

        # ---------- k projection (feature-major) -> AllGather ----------
        def proj_feat(w_t, bias_col0, out_tag=None, to_dram=None):
            res = []
            for oc in range(KT):
                wblk = wpool.tile([128, D], bf16, tag="wblk", bufs=3)
                nc.sync.dma_start(wblk[:], w_t[oc])
                ps = psum.tile([128, TOK], f32, tag="proj")
                for k in range(KT):
                    nc.tensor.matmul(
                        ps[:], lhsT=wblk[:, k * 128:(k + 1) * 128],
                        rhs=h1T[k][:], start=(k == 0), stop=(k == KT - 1))
                if out_tag is not None:
                    r = persist.tile([128, TOK], bf16, tag=f"{out_tag}{oc}",
                                     name=f"{out_tag}{oc}")
                else:
                    r = trans.tile([128, TOK], bf16, tag="proj_evac",
                                   name="proj_evac")
                nc.vector.tensor_scalar(
                    out=r[:], in0=ps[:], scalar1=bcol(bias_col0 + oc),
                    scalar2=None, op0=OP.add)
                if to_dram is not None:
                    nc.sync.dma_start(to_dram[oc * 128:(oc + 1) * 128, :], r[:])
                res.append(r)
            return res

        proj_feat(wk_t, 56, to_dram=k_ag_in)
        nc.gpsimd.collective_compute(
            "AllGather", OP.bypass, replica_groups=PAIRS,
            ins=[k_ag_in[:].opt()], outs=[k_ag_out[:].opt()])

        # ---------- v projection (token-major) -> AllGather ----------
        wv_tiles = []
        for k in range(KT):
            wblk = wpool.tile([128, D], bf16, tag="wvblk", name="wvblk",
                              bufs=8)
            nc.sync.dma_start(wblk[:], wvT[k * 128:(k + 1) * 128, :])
            wv_tiles.append(wblk)
        for tb in range(4):
            for half in range(2):
                ps = psum.tile([128, TOK], f32, tag="proj")
                for k in range(KT):
                    nc.tensor.matmul(
                        ps[:], lhsT=h1T[k][:, tb * 128:(tb + 1) * 128],
                        rhs=wv_tiles[k][:, half * 512:(half + 1) * 512],
                        start=(k == 0), stop=False)
                nc.tensor.matmul(
                    ps[:], lhsT=ones_m[:],
                    rhs=bvr_sb[:, half * 512:(half + 1) * 512],
                    start=False, stop=True)
                vtile = trans.tile([128, TOK], bf16, tag="v_evac",
                                   name="v_evac")
                nc.vector.tensor_copy(out=vtile[:], in_=ps[:])
                nc.sync.dma_start(
                    v_ag_in[tb * 128:(tb + 1) * 128,
                            half * 512:(half + 1) * 512], vtile[:])
        nc.gpsimd.collective_compute(
            "AllGather", OP.bypass, replica_groups=PAIRS,
            ins=[v_ag_in[:].opt()], outs=[v_ag_out[:].opt()])

        # ---------- q projection ----------
        qT = proj_feat(wq_t, 48, out_tag="qT")

        if DEBUG:
            for _oc in range(KT):
                dump(dbg_qT[_oc * 128:(_oc + 1) * 128, :], qT[_oc])
            _kt = trans.tile([128, TOK], bf16, tag="dbgk", bufs=2)
            for _blk in range(16):
                nc.sync.dma_start(
                    _kt[:], k_ag_out[_blk * 128:(_blk + 1) * 128, :])
                dump(dbg_kago[_blk * 128:(_blk + 1) * 128, :], _kt)
            _vt = trans.tile([128, D], bf16, tag="dbgv", bufs=2)
            for _blk in range(8):
                nc.sync.dma_start(
                    _vt[:], v_ag_out[_blk * 128:(_blk + 1) * 128, :])
                dump(dbg_vago[_blk * 128:(_blk + 1) * 128, :], _vt)

        # ---------- rest of adaLN: emitted in chunks inside the head
        # loop so the matmuls fill PE gaps during the exp-bound phase ----
        ps_mod2 = psum.tile([128, 32], f32, tag="proj", name="ps_mod2")

        def emit_mod_chunk(i):
            part, k = i // 8, i % 8
            col0 = 16 + part * 8
            wt = wpool.tile([128, 1024], bf16, tag="wadas", bufs=2,
                            name="wadar")
            nc.sync.dma_start(
                wt[:], wadaT[k * 128:(k + 1) * 128,
                             col0 * 128:(col0 + 8) * 128])
            for j in range(8):
                nc.tensor.matmul(
                    ps_mod2[:, part * 8 + j:part * 8 + j + 1],
                    lhsT=wt[:, j * 128:(j + 1) * 128],
                    rhs=silu_sb[:, k:k + 1],
                    start=(i == 0 and j == 0), stop=(k == KT - 1),
                    skip_group_check=True)

        # ---------- attention ----------
        attnT = []
        for hp in range(KT):
            attnT.append(persist.tile([128, TOK], bf16, tag=f"attnT{hp}",
                                      name=f"attnT{hp}"))
        dn16 = consts.tile([16, TOK], bf16)
        SCALE = 1.0 / 8.0

        for hp in range(KT):
            kT_hp = kv_pool.tile([128, S], bf16, tag="kT_hp")
            for blk in range(2):
                nc.gpsimd.dma_start(
                    kT_hp[:, blk * 512:(blk + 1) * 512],
                    k_ag_out[blk * D + hp * 128:blk * D + (hp + 1) * 128, :])
            v_AB = []
            pT_AB = []
            for hh in range(2):
                h = 2 * hp + hh
                v_h = kv_pool.tile([128, 8, 65], bf16, tag="v_h")
                nc.gpsimd.dma_start(
                    v_h[:, :, 0:64],
                    v_ag_out[:, h * 64:(h + 1) * 64].rearrange(
                        "(kc p) c -> p kc c", p=128))
                nc.vector.memset(v_h[:, :, 64:65], 1.0)
                v_AB.append(v_h)
                pT_AB.append(pT_pool.tile([128, 8 * TOK], bf16, tag="pT",
                                          name=f"pT{hh}"))
            # row-packed scores: both heads of the pair run concurrently in
            # the PE array (rows 0-63 / 64-127), separate PSUM banks
            for g in range(4):
                ps_AB = [psum.tile([128, 2 * TOK], f32, tag="big",
                                   name=f"ps_s{hh}") for hh in range(2)]
                for i in range(2):
                    kc = 2 * g + i
                    for hh in range(2):
                        nc.tensor.matmul(
                            ps_AB[hh][:, i * TOK:(i + 1) * TOK],
                            lhsT=kT_hp[hh * 64:(hh + 1) * 64,
                                       kc * 128:(kc + 1) * 128],
                            rhs=qT[hp][hh * 64:(hh + 1) * 64, :],
                            start=True, stop=True)
                for hh in range(2):
                    nc.scalar.activation(
                        out=pT_AB[hh][:, g * 1024:(g + 1) * 1024],
                        in_=ps_AB[hh][:], func=AF.Exp, scale=SCALE)
            for hh in range(2):
                h = 2 * hp + hh
                ps_av = psum.tile([128, TOK], f32, tag="av")
                for kc in range(8):
                    nc.tensor.matmul(
                        ps_av[0:65, :], lhsT=v_AB[hh][:, kc, :],
                        rhs=pT_AB[hh][:, kc * TOK:(kc + 1) * TOK],
                        start=(kc == 0), stop=(kc == 7))
                nc.vector.tensor_copy(out=attnT[hp][hh * 64:(hh + 1) * 64, :],
                                      in_=ps_av[0:64, :])
                dn = trans.tile([1, TOK], bf16, tag="dn", bufs=2, name="dn")
                nc.vector.tensor_copy(out=dn[:], in_=ps_av[64:65, :])
                nc.gpsimd.dma_start(dn16[h:h + 1, :], dn[:])
                emit_mod_chunk(4 * hp + 2 * hh)
                emit_mod_chunk(4 * hp + 2 * hh + 1)

        # batched reciprocal of all 16 denominators, then normalize in place
        rd16 = consts.tile([16, TOK], bf16)
        with nc.allow_low_precision(reason="softmax denom recip in bf16"):
            nc.vector.reciprocal(rd16[:], dn16[:])
        for h in range(H):
            hp, hh = h // 2, h % 2
            psb = psum.tile([128, TOK], f32, tag="av")
            nc.tensor.matmul(psb[0:64, :],
                             lhsT=onehot[:, h * 64:(h + 1) * 64],
                             rhs=rd16[:], start=True, stop=True)
            nc.vector.tensor_tensor(
                out=attnT[hp][hh * 64:(hh + 1) * 64, :],
                in0=attnT[hp][hh * 64:(hh + 1) * 64, :],
                in1=psb[0:64, :], op=OP.mult)

        # evacuate the interleaved mod-rest accumulation
        nc.vector.tensor_tensor(out=mod_sb[:, 16:48], in0=ps_mod2[:],
                                in1=bias_sb[:, 16:48], op=OP.add)
        nc.vector.tensor_scalar_add(mod1p_sb[:, 16:48], mod_sb[:, 16:48], 1.0)
        if DEBUG:
            dump(dbg_mod[:], mod_sb)
            for _hp in range(KT):
                dump(dbg_attnT[_hp * 128:(_hp + 1) * 128, :], attnT[_hp])

        # ---------- Wo + residual1 (feature-major) ----------
        x2T = []
        for dc in range(KT):
            wblk = wpool.tile([128, D], bf16, tag="wblk", bufs=3)
            nc.sync.dma_start(wblk[:], wo_t[dc])
            ps_y = psum.tile([128, TOK], f32, tag="proj")
            for k in range(KT):
                nc.tensor.matmul(ps_y[:], lhsT=wblk[:, k * 128:(k + 1) * 128],
                                 rhs=attnT[k][:],
                                 start=(k == 0), stop=(k == KT - 1))
            ysc = trans.tile([128, TOK], f32, tag="sc_evac", name="ysc")
            nc.vector.tensor_scalar(
                out=ysc[:], in0=ps_y[:], scalar1=bcol(64 + dc),
                scalar2=mod_sb[:, 16 + dc:17 + dc], op0=OP.add, op1=OP.mult)
            x2t = persist.tile([128, TOK], f32, tag=f"x2T{dc}", name=f"x2T{dc}")
            nc.vector.tensor_tensor(out=x2t[:], in0=ysc[:],
                                    in1=xfeat_sb[dc][:], op=OP.add)
            x2T.append(x2t)
            if DEBUG:
                dump(dbg_x2T[dc * 128:(dc + 1) * 128, :], x2t)

        # ---------- LN2 + modulate ----------
        layer_norm_mod_feat(x2T, 32, 24, "h2")
        h2T = [hT_tiles[("h2", d)] for d in range(KT)]

        # ---------- MLP (token-local, two token-halves) ----------
        for half in range(2):
            tsl = slice(half * HT, (half + 1) * HT)
            G_sb = []
            for g4 in range(8):  # groups of 4 HID blocks
                w1q = wpool.tile([128, 8 * 512], bf16, tag="w1q", bufs=2)
                nc.sync.dma_start(w1q[:], w1_t[g4])
                ps_g = psum.tile([128, 4 * HT], f32, tag="big")
                for j in range(4):
                    hc = 4 * g4 + j
                    for k in range(KT):
                        nc.tensor.matmul(
                            ps_g[:, j * HT:(j + 1) * HT],
                            lhsT=w1q[:, k * 512 + j * 128:k * 512 + (j + 1) * 128],
                            rhs=h2T[k][:, tsl],
                            start=(k == 0 and j % 2 == 0),
                            stop=False, skip_group_check=True)
                    nc.tensor.matmul(
                        ps_g[:, j * HT:(j + 1) * HT],
                        lhsT=b1r_sb[:, hc * 128:(hc + 1) * 128],
                        rhs=ones_tok[:], start=False,
                        stop=(j % 2 == 1), skip_group_check=True)
                gt = persist.tile([128, 4 * HT], bf16, tag=f"G{g4}",
                                  name=f"G{g4}", bufs=1)
                nc.scalar.activation(out=gt[:], in_=ps_g[:], func=AF.Gelu)
                G_sb.append(gt)

            for dc in range(KT):
                ps_z = psum.tile([128, HT], f32, tag="proj")
                for kg2 in range(2):  # two [128, 2048] weight slabs
                    wblk = wpool.tile([128, 2048], bf16, tag="w2blk", bufs=2)
                    nc.sync.dma_start(wblk[:], w2_t[dc, kg2])
                    for i in range(16):
                        kb = 16 * kg2 + i
                        nc.tensor.matmul(
                            ps_z[:], lhsT=wblk[:, i * 128:(i + 1) * 128],
                            rhs=G_sb[kb // 4][:, (kb % 4) * HT:(kb % 4 + 1) * HT],
                            start=(kb == 0), stop=(kb == HC - 1))
                zsc = trans.tile([128, HT], f32, tag="sc_evac", name="zsc")
                nc.vector.tensor_scalar(
                    out=zsc[:], in0=ps_z[:], scalar1=bcol(72 + dc),
                    scalar2=mod_sb[:, 40 + dc:41 + dc],
                    op0=OP.add, op1=OP.mult)
                ot = trans.tile([128, HT], f32, tag="sc_evac", name="ot")
                nc.vector.tensor_tensor(out=ot[:], in0=zsc[:],
                                        in1=x2T[dc][:, tsl], op=OP.add)
                nc.gpsimd.dma_start(out_feat[dc * 128:(dc + 1) * 128, tsl],
                                    ot[:])

        ctx.close()

    nc.compile()
    return nc


def _pack_bias(bq, bk, bo, b2, bada):
    t = np.zeros((128, 80), np.float32)
    t[:, 0:48] = bada.reshape(48, 128).T
    t[:, 48:56] = bq.reshape(8, 128).T
    t[:, 56:64] = bk.reshape(8, 128).T
    t[:, 64:72] = bo.reshape(8, 128).T
    t[:, 72:80] = b2.reshape(8, 128).T
    return t


def _slab_oc(wT):
    """[D, D] W.T -> [8, 128, 1024]: slab[oc][p][k*128+c] = wT[k*128+p, oc*128+c]"""
    w = wT.reshape(KT, 128, KT, 128)          # [k, p, oc, c]
    return np.ascontiguousarray(w.transpose(2, 1, 0, 3).reshape(KT, 128, D))


def _slab_w1(w1T):
    """[D, HID] W1.T -> [8(g4), 128(p), 8(k)*512]: slab[g4][p][k*512+c] =
    w1T[k*128+p, g4*512+c]"""
    w = w1T.reshape(KT, 128, 8, 512)          # [k, p, g4, c]
    return np.ascontiguousarray(w.transpose(2, 1, 0, 3).reshape(KT, 128, 8 * 512))


def _slab_w2(w2T):
    """[HID, D] W2.T -> [8(dc), 2(kg2), 128(p), 16(i)*128]: slab[dc,kg2,p,i*128+c]
    = w2T[(16*kg2+i)*128+p, dc*128+c]"""
    w = w2T.reshape(2, 16, 128, KT, 128)      # [kg2, i, p, dc, c]
    return np.ascontiguousarray(
        w.transpose(3, 0, 2, 1, 4).reshape(KT, 2, 128, 2048))


def kernel(x, cond, Wq, bq, Wk, bk, Wv, bv, Wo, bo, W1, b1, W2, b2, Wada, bada):
    import ml_dtypes
    from concourse.bass_utils import run_bass_kernel_spmd

    bf = ml_dtypes.bfloat16
    if "nc" not in _cached:
        _cached["nc"] = _build()
    nc = _cached["nc"]

    x = np.asarray(x, np.float32)
    cond = np.asarray(cond, np.float32)
    to_bf_T = lambda w: np.ascontiguousarray(
        np.asarray(w, np.float32).T).astype(bf)
    wq_t = _slab_oc(np.asarray(Wq, np.float32).T).astype(bf)
    wk_t = _slab_oc(np.asarray(Wk, np.float32).T).astype(bf)
    wo_t = _slab_oc(np.asarray(Wo, np.float32).T).astype(bf)
    wvT = to_bf_T(Wv)
    w1_t = _slab_w1(np.asarray(W1, np.float32).T).astype(bf)
    w2_t = _slab_w2(np.asarray(W2, np.float32).T).astype(bf)
    wadaT = to_bf_T(Wada)
    biasc = _pack_bias(np.asarray(bq, np.float32), np.asarray(bk, np.float32),
                       np.asarray(bo, np.float32), np.asarray(b2, np.float32),
                       np.asarray(bada, np.float32))
    bv_row = np.asarray(bv, np.float32).reshape(1, D).astype(bf)
    onehot = np.repeat(np.eye(16, dtype=np.float32), 64, axis=1).astype(bf)

    in_maps = []
    for c in range(N_CORES):
        b, h = c // 2, c % 2
        xs = x[b, h * TOK:(h + 1) * TOK, :]
        in_maps.append({
            "x_feat": np.ascontiguousarray(xs.T),
            "condT": np.ascontiguousarray(cond[b, 0].reshape(8, 128).T),
            "wq_t": wq_t, "wk_t": wk_t, "wo_t": wo_t, "wvT": wvT,
            "w1_t": w1_t, "w2_t": w2_t, "wadaT": wadaT,
            "biasc": biasc, "bv_row": bv_row,
            "b1_row": np.asarray(b1, np.float32).reshape(1, HID).astype(bf),
            "onehot_d": onehot,
        })

    _cached["in_maps"] = in_maps
    res = run_bass_kernel_spmd(nc, in_maps, core_ids=list(range(N_CORES)))
    _cached["results"] = res.results
    out = np.empty((B, S, D), np.float32)
    for c in range(N_CORES):
        b, h = c // 2, c % 2
        out[b, h * TOK:(h + 1) * TOK, :] = res.results[c]["out_feat"].T
    return out


# revision 23
# speedup vs baseline: 1.1606x; 1.1606x over previous
"""AdaLN DiT block on 8 Trainium2 NeuronCores.

Sharding: core c owns tokens [h*512,(h+1)*512) of batch b, where b=c//2,
h=c%2. Attention is sharded-Q: each core computes q/k/v for its own 512
tokens with the full projection weights, pair-AllGathers kT and v so it
has the full-sequence keys/values of its batch, then runs all 16 heads
for its own 512 query tokens. Wo and the whole MLP are token-local with
full (bf16, host-pre-transposed) weights streamed from HBM. The adaLN
modulation vector is computed per-core for its own batch. No reduce
collectives; the only comm is the per-pair kT/v AllGather.

Everything on-chip is feature-major ([D-on-partitions, tokens]); LN
statistics are computed with ones-vector matmuls on the TensorEngine
(partition-dim reduction), so no layout transposes are needed anywhere.
Matmuls run in bf16 (fp32 PSUM accumulation); the residual stream stays
fp32.

PSUM budget (8 banks): tag "big" [128,1024] x2 = 4 banks (scores, fc1),
tag "proj" [128,512] x2 = 2 (projections, LN stats/bcasts), tag "av"
[128,512] x2 = 2 (adaLN accum early, attention AV / denom-bcast later).
NOTE: matmul start=True clears the WHOLE psum bank, so only the first
matmul touching a bank may set it; later column-groups rely on the
per-element has_written bits for first-touch overwrite.
"""

import numpy as np

B, S, D, H, HID = 4, 1024, 1024, 16, 4096
DK = D // H  # 64
N_CORES = 8
TOK = 512
HT = TOK // 2
EPS = 1e-6
KT = 8    # 128-row blocks in D
HC = 32   # 128-row blocks in HID

_cached = {}
DEBUG = False


def _build():
    import contextlib
    import concourse.bass as bass  # noqa: F401
    import concourse.tile as tile
    from concourse import bacc, mybir

    f32 = mybir.dt.float32
    bf16 = mybir.dt.bfloat16
    AF = mybir.ActivationFunctionType
    OP = mybir.AluOpType

    nc = bacc.Bacc("TRN2", target_bir_lowering=False, debug=False,
                   num_devices=N_CORES)

    # ---- per-core external I/O ----
    x_feat = nc.dram_tensor("x_feat", [D, TOK], f32, kind="ExternalInput")
    condT = nc.dram_tensor("condT", [128, 8], f32, kind="ExternalInput")
    wq_t = nc.dram_tensor("wq_t", [KT, 128, D], bf16, kind="ExternalInput")
    wk_t = nc.dram_tensor("wk_t", [KT, 128, D], bf16, kind="ExternalInput")
    wo_t = nc.dram_tensor("wo_t", [KT, 128, D], bf16, kind="ExternalInput")
    wvT = nc.dram_tensor("wvT", [D, D], bf16, kind="ExternalInput")
    w1_t = nc.dram_tensor("w1_t", [KT, 128, 8 * 512], bf16, kind="ExternalInput")
    w2_t = nc.dram_tensor("w2_t", [KT, 2, 128, 2048], bf16, kind="ExternalInput")
    wadaT = nc.dram_tensor("wadaT", [D, 6 * D], bf16, kind="ExternalInput")
    # packed per-partition bias columns (fp32): 0..47 bada, 48..55 bq,
    # 56..63 bk, 64..71 bo, 72..79 b2
    biasc = nc.dram_tensor("biasc", [128, 80], f32, kind="ExternalInput")
    bv_row = nc.dram_tensor("bv_row", [1, D], bf16, kind="ExternalInput")
    b1_row = nc.dram_tensor("b1_row", [1, HID], bf16, kind="ExternalInput")
    onehot_d = nc.dram_tensor("onehot_d", [16, 1024], bf16, kind="ExternalInput")
    out_feat = nc.dram_tensor("out_feat", [D, TOK], f32, kind="ExternalOutput")
    if DEBUG:
        dbg_mod = nc.dram_tensor("dbg_mod", [128, 48], f32, kind="ExternalOutput")
        dbg_h1T = nc.dram_tensor("dbg_h1T", [D, TOK], f32, kind="ExternalOutput")
        dbg_qT = nc.dram_tensor("dbg_qT", [D, TOK], f32, kind="ExternalOutput")
        dbg_kago = nc.dram_tensor("dbg_kago", [2 * D, TOK], f32, kind="ExternalOutput")
        dbg_vago = nc.dram_tensor("dbg_vago", [2 * TOK, D], f32, kind="ExternalOutput")
        dbg_attnT = nc.dram_tensor("dbg_attnT", [D, TOK], f32, kind="ExternalOutput")
        dbg_x2T = nc.dram_tensor("dbg_x2T", [D, TOK], f32, kind="ExternalOutput")
        dbg_h2T = nc.dram_tensor("dbg_h2T", [D, TOK], f32, kind="ExternalOutput")

    PAIRS = [[2 * i, 2 * i + 1] for i in range(B)]

    with tile.TileContext(nc) as tc:
        ctx = contextlib.ExitStack()
        consts = ctx.enter_context(tc.tile_pool(name="consts", bufs=1))
        persist = ctx.enter_context(tc.tile_pool(name="persist", bufs=1))
        wpool = ctx.enter_context(tc.tile_pool(name="wpool", bufs=3))
        trans = ctx.enter_context(tc.tile_pool(name="trans", bufs=3))
        pT_pool = ctx.enter_context(tc.tile_pool(name="pTp", bufs=2))
        kv_pool = ctx.enter_context(tc.tile_pool(name="kvp", bufs=2))
        dram = ctx.enter_context(tc.tile_pool(name="dram", bufs=1, space="DRAM"))
        psum = ctx.enter_context(tc.tile_pool(name="psum", bufs=2, space="PSUM"))

        # ---- internal DRAM for the pair AllGather ----
        k_ag_in = dram.tile([D, TOK], bf16, tag="k_ag_in")
        k_ag_out = dram.tile([2 * D, TOK], bf16, tag="k_ag_out")
        v_ag_in = dram.tile([TOK, D], bf16, tag="v_ag_in")
        v_ag_out = dram.tile([2 * TOK, D], bf16, tag="v_ag_out")

        # ---------- constants ----------
        bias_sb = consts.tile([128, 80], f32)
        nc.gpsimd.dma_start(bias_sb[:], biasc[:])
        cond_sb = consts.tile([128, 8], f32)
        nc.gpsimd.dma_start(cond_sb[:], condT[:])
        bvr_sb = consts.tile([1, D], bf16)
        nc.gpsimd.dma_start(bvr_sb[:], bv_row[:])
        b1r_sb = consts.tile([1, HID], bf16)
        nc.gpsimd.dma_start(b1r_sb[:], b1_row[:])
        onehot = consts.tile([16, 1024], bf16)
        nc.gpsimd.dma_start(onehot[:], onehot_d[:])
        eps_sb = consts.tile([1, 1], f32)
        nc.vector.memset(eps_sb[:], EPS)
        ones_m = consts.tile([1, 128], bf16)
        nc.vector.memset(ones_m[:], 1.0)
        ones_col = consts.tile([128, 1], bf16)
        nc.vector.memset(ones_col[:], 1.0)
        ones_tok = consts.tile([1, HT], bf16)
        nc.vector.memset(ones_tok[:], 1.0)

        def bcol(i):
            return bias_sb[:, i:i + 1]

        def dump(dst_slice, src_tile):
            if DEBUG:
                tmp = trans.tile(list(src_tile.shape), f32, tag="dbg",
                                 name="dbgtmp", bufs=2)
                nc.vector.tensor_copy(out=tmp[:], in_=src_tile[:])
                nc.sync.dma_start(dst_slice, tmp[:])

        # ---------- adaLN modulation (own batch) ----------
        silu_sb = consts.tile([128, 8], bf16)
        nc.scalar.activation(silu_sb[:], cond_sb[:], AF.Silu)

        mod_sb = consts.tile([128, 48], f32)
        mod1p_sb = consts.tile([128, 48], f32)

        def mod_part(ps, col0, ncols, bank_first=True):
            for k in range(KT):
                wt = wpool.tile([128, ncols * 128], bf16, tag="wadas", bufs=2)
                nc.sync.dma_start(
                    wt[:], wadaT[k * 128:(k + 1) * 128,
                                 col0 * 128:(col0 + ncols) * 128])
                for j in range(ncols):
                    nc.tensor.matmul(
                        ps[:, j:j + 1], lhsT=wt[:, j * 128:(j + 1) * 128],
                        rhs=silu_sb[:, k:k + 1],
                        start=(bank_first and k == 0 and j == 0),
                        stop=(k == KT - 1), skip_group_check=True)
            return ps

        # urgent: gama1 (blocks 0..7), beta1 (8..15)
        ps_mod = psum.tile([128, 16], f32, tag="av", bufs=2)
        mod_part(ps_mod[:, 0:8], 0, 8, bank_first=True)
        mod_part(ps_mod[:, 8:16], 8, 8, bank_first=False)
        nc.vector.tensor_tensor(out=mod_sb[:, 0:16], in0=ps_mod[:],
                                in1=bias_sb[:, 0:16], op=OP.add)
        nc.vector.tensor_scalar_add(mod1p_sb[:, 0:16], mod_sb[:, 0:16], 1.0)

        # ---------- feature-major LayerNorm via PE statistics ----------
        xfeat_sb = []
        for d in range(KT):
            xf = persist.tile([128, TOK], f32, tag=f"xfeat{d}", name=f"xf{d}")
            nc.gpsimd.dma_start(xf[:], x_feat[d * 128:(d + 1) * 128, :])
            xfeat_sb.append(xf)

        hT_tiles = {}

        def layer_norm_mod_feat(src_tiles, beta_blk, gama_blk, key):
            """src: 8 fp32 [128, TOK] feature-major tiles ->
            hT = LN(src)*(1+beta)+gama, bf16 feature-major (persist)."""
            ps_sum = psum.tile([1, TOK], f32, tag="proj", name="ps_sum")
            ps_sq = psum.tile([1, TOK], f32, tag="proj", name="ps_sq")
            for d in range(KT):
                cb = trans.tile([128, TOK], bf16, tag="lncast", bufs=3,
                                name="lncast")
                nc.vector.tensor_copy(out=cb[:], in_=src_tiles[d][:])
                nc.tensor.matmul(ps_sum[:], lhsT=ones_col[:], rhs=cb[:],
                                 start=(d == 0), stop=(d == KT - 1))
                sq = trans.tile([128, TOK], bf16, tag="lnsq", bufs=3,
                                name="lnsq")
                nc.vector.tensor_tensor(out=sq[:], in0=cb[:],
                                        in1=cb[:], op=OP.mult)
                nc.tensor.matmul(ps_sq[:], lhsT=ones_col[:], rhs=sq[:],
                                 start=(d == 0), stop=(d == KT - 1))
            # stats in one [1, 5*TOK] fp32 tile: mu|ex2|var|std|rstd
            st = trans.tile([1, 5 * TOK], f32, tag="lnstat", name="lnstat",
                            bufs=2)
            mu, ex2 = st[:, 0:TOK], st[:, TOK:2 * TOK]
            var, std = st[:, 2 * TOK:3 * TOK], st[:, 3 * TOK:4 * TOK]
            rstd = st[:, 4 * TOK:5 * TOK]
            nc.vector.tensor_scalar(out=mu, in0=ps_sum[:], scalar1=1.0 / D,
                                    scalar2=None, op0=OP.mult)
            nc.vector.tensor_scalar(out=ex2, in0=ps_sq[:], scalar1=1.0 / D,
                                    scalar2=None, op0=OP.mult)
            nc.vector.tensor_tensor(out=var, in0=mu, in1=mu, op=OP.mult)
            nc.vector.tensor_tensor(out=var, in0=ex2, in1=var, op=OP.subtract)
            nc.scalar.activation(std, var, AF.Sqrt, bias=eps_sb[:], scale=1.0)
            nc.vector.reciprocal(rstd, std)
            stbf = trans.tile([1, 2 * TOK], bf16, tag="lnstatbf",
                              name="lnstatbf", bufs=2)
            mu_bf, rstd_bf = stbf[:, 0:TOK], stbf[:, TOK:2 * TOK]
            nc.vector.tensor_copy(out=mu_bf, in_=mu)
            nc.vector.tensor_copy(out=rstd_bf, in_=rstd)
            # broadcast mu and rstd across partitions via ones-matmul
            ps_mu = psum.tile([128, TOK], f32, tag="proj", name="ps_mu")
            nc.tensor.matmul(ps_mu[:], lhsT=ones_m[:], rhs=mu_bf,
                             start=True, stop=True)
            ps_rstd = psum.tile([128, TOK], f32, tag="proj", name="ps_rstd")
            nc.tensor.matmul(ps_rstd[:], lhsT=ones_m[:], rhs=rstd_bf,
                             start=True, stop=True)
            for d in range(KT):
                t1 = trans.tile([128, TOK], f32, tag="lnt", bufs=2,
                                name="lnt1")
                nc.vector.tensor_tensor(out=t1[:], in0=src_tiles[d][:],
                                        in1=ps_mu[:], op=OP.subtract)
                t2 = trans.tile([128, TOK], f32, tag="lnt", bufs=2,
                                name="lnt2")
                nc.vector.tensor_tensor(out=t2[:], in0=t1[:], in1=ps_rstd[:],
                                        op=OP.mult)
                ht = persist.tile([128, TOK], bf16, tag="hT", bufs=8,
                                  name="hT")
                nc.vector.tensor_scalar(
                    out=ht[:], in0=t2[:],
                    scalar1=mod1p_sb[:, beta_blk + d:beta_blk + d + 1],
                    scalar2=mod_sb[:, gama_blk + d:gama_blk + d + 1],
                    op0=OP.mult, op1=OP.add)
                hT_tiles[(key, d)] = ht
                if DEBUG:
                    dump((dbg_h1T if key == "h1" else dbg_h2T)
                         [d * 128:(d + 1) * 128, :], ht)

        layer_norm_mod_feat(xfeat_sb, 8, 0, "h1")
        h1T = [hT_tiles[("h1", d)] for d in range(KT)]

        # ---------- k projection (feature-major) -> AllGather ----------
        def proj_feat(w_t, bias_col0, out_tag=None, to_dram=None):
            res = []
            for oc in range(KT):
                wblk = wpool.tile([128, D], bf16, tag="wblk", bufs=3)
                nc.sync.dma_start(wblk[:], w_t[oc])
                ps = psum.tile([128, TOK], f32, tag="proj")
                for k in range(KT):
                    nc.tensor.matmul(
                        ps[:], lhsT=wblk[:, k * 128:(k + 1) * 128],
                        rhs=h1T[k][:], start=(k == 0), stop=(k == KT - 1))
                if out_tag is not None:
                    r = persist.tile([128, TOK], bf16, tag=f"{out_tag}{oc}",
                                     name=f"{out_tag}{oc}")
                else:
                    r = trans.tile([128, TOK], bf16, tag="proj_evac",
                                   name="proj_evac")
                nc.vector.tensor_scalar(
                    out=r[:], in0=ps[:], scalar1=bcol(bias_col0 + oc),
                    scalar2=None, op0=OP.add)
                if to_dram is not None:
                    nc.sync.dma_start(to_dram[oc * 128:(oc + 1) * 128, :], r[:])
                res.append(r)
            return res

        proj_feat(wk_t, 56, to_dram=k_ag_in)
        nc.gpsimd.collective_compute(
            "AllGather", OP.bypass, replica_groups=PAIRS,
            ins=[k_ag_in[:].opt()], outs=[k_ag_out[:].opt()])

        # ---------- v projection (token-major) -> AllGather ----------
        wv_tiles = []
        for k in range(KT):
            wblk = wpool.tile([128, D], bf16, tag="wvblk", name="wvblk",
                              bufs=8)
            nc.sync.dma_start(wblk[:], wvT[k * 128:(k + 1) * 128, :])
            wv_tiles.append(wblk)
        for tb in range(4):
            for half in range(2):
                ps = psum.tile([128, TOK], f32, tag="proj")
                for k in range(KT):
                    nc.tensor.matmul(
                        ps[:], lhsT=h1T[k][:, tb * 128:(tb + 1) * 128],
                        rhs=wv_tiles[k][:, half * 512:(half + 1) * 512],
                        start=(k == 0), stop=False)
                nc.tensor.matmul(
                    ps[:], lhsT=ones_m[:],
                    rhs=bvr_sb[:, half * 512:(half + 1) * 512],
                    start=False, stop=True)
                vtile = trans.tile([128, TOK], bf16, tag="v_evac",
                                   name="v_evac")
                nc.vector.tensor_copy(out=vtile[:], in_=ps[:])
                nc.sync.dma_start(
                    v_ag_in[tb * 128:(tb + 1) * 128,
                            half * 512:(half + 1) * 512], vtile[:])
        nc.gpsimd.collective_compute(
            "AllGather", OP.bypass, replica_groups=PAIRS,
            ins=[v_ag_in[:].opt()], outs=[v_ag_out[:].opt()])

        # ---------- q projection ----------
        qT = proj_feat(wq_t, 48, out_tag="qT")

        if DEBUG:
            for _oc in range(KT):
                dump(dbg_qT[_oc * 128:(_oc + 1) * 128, :], qT[_oc])
            _kt = trans.tile([128, TOK], bf16, tag="dbgk", bufs=2)
            for _blk in range(16):
                nc.sync.dma_start(
                    _kt[:], k_ag_out[_blk * 128:(_blk + 1) * 128, :])
                dump(dbg_kago[_blk * 128:(_blk + 1) * 128, :], _kt)
            _vt = trans.tile([128, D], bf16, tag="dbgv", bufs=2)
            for _blk in range(8):
                nc.sync.dma_start(
                    _vt[:], v_ag_out[_blk * 128:(_blk + 1) * 128, :])
                dump(dbg_vago[_blk * 128:(_blk + 1) * 128, :], _vt)

        # ---------- rest of adaLN: emitted in chunks inside the head
        # loop so the matmuls fill PE gaps during the exp-bound phase ----
        ps_mod2 = psum.tile([128, 32], f32, tag="proj", name="ps_mod2")

        def emit_mod_chunk(i):
            part, k = i // 8, i % 8
            col0 = 16 + part * 8
            wt = wpool.tile([128, 1024], bf16, tag="wadas", bufs=2,
                            name="wadar")
            nc.sync.dma_start(
                wt[:], wadaT[k * 128:(k + 1) * 128,
                             col0 * 128:(col0 + 8) * 128])
            for j in range(8):
                nc.tensor.matmul(
                    ps_mod2[:, part * 8 + j:part * 8 + j + 1],
                    lhsT=wt[:, j * 128:(j + 1) * 128],
                    rhs=silu_sb[:, k:k + 1],
                    start=(i == 0 and j == 0), stop=(k == KT - 1),
                    skip_group_check=True)

        # ---------- attention ----------
        attnT = []
        for hp in range(KT):
            attnT.append(persist.tile([128, TOK], bf16, tag=f"attnT{hp}",
                                      name=f"attnT{hp}"))
        dn16 = consts.tile([16, TOK], bf16)
        SCALE = 1.0 / 8.0

        for hp in range(KT):
            kT_hp = kv_pool.tile([128, S], bf16, tag="kT_hp")
            for blk in range(2):
                nc.sync.dma_start(
                    kT_hp[:, blk * 512:(blk + 1) * 512],
                    k_ag_out[blk * D + hp * 128:blk * D + (hp + 1) * 128, :])
            v_AB = []
            for hh in range(2):
                h = 2 * hp + hh
                v_h = kv_pool.tile([128, 8, 65], bf16, tag="v_h", bufs=4)
                nc.sync.dma_start(
                    v_h[:, :, 0:64],
                    v_ag_out[:, h * 64:(h + 1) * 64].rearrange(
                        "(kc p) c -> p kc c", p=128))
                nc.vector.memset(v_h[:, :, 64:65], 1.0)
                v_AB.append(v_h)
            # row-packed scores: both heads of the pair run concurrently in
            # the PE array (rows 0-63 / 64-127), separate PSUM banks.
            # pT tiles are per (head, group) for fine-grained rotation.
            pT_g = {}
            for g in range(4):
                ps_AB = [psum.tile([128, 2 * TOK], f32, tag="big",
                                   name=f"ps_s{hh}") for hh in range(2)]
                for i in range(2):
                    kc = 2 * g + i
                    for hh in range(2):
                        nc.tensor.matmul(
                            ps_AB[hh][:, i * TOK:(i + 1) * TOK],
                            lhsT=kT_hp[hh * 64:(hh + 1) * 64,
                                       kc * 128:(kc + 1) * 128],
                            rhs=qT[hp][hh * 64:(hh + 1) * 64, :],
                            start=True, stop=True)
                for hh in range(2):
                    pt = pT_pool.tile([128, 2 * TOK], bf16, tag="pT",
                                      name="pTg", bufs=6)
                    nc.scalar.activation(out=pt[:], in_=ps_AB[hh][:],
                                         func=AF.Exp, scale=SCALE)
                    pT_g[(hh, g)] = pt
            for hh in range(2):
                h = 2 * hp + hh
                ps_av = psum.tile([128, TOK], f32, tag="av")
                for kc in range(8):
                    nc.tensor.matmul(
                        ps_av[0:65, :], lhsT=v_AB[hh][:, kc, :],
                        rhs=pT_g[(hh, kc // 2)][:, (kc % 2) * TOK:
                                                (kc % 2 + 1) * TOK],
                        start=(kc == 0), stop=(kc == 7))
                nc.vector.tensor_copy(out=attnT[hp][hh * 64:(hh + 1) * 64, :],
                                      in_=ps_av[0:64, :])
                dn = trans.tile([1, TOK], bf16, tag="dn", bufs=2, name="dn")
                nc.vector.tensor_copy(out=dn[:], in_=ps_av[64:65, :])
                nc.gpsimd.dma_start(dn16[h:h + 1, :], dn[:])
                emit_mod_chunk(4 * hp + 2 * hh)
                emit_mod_chunk(4 * hp + 2 * hh + 1)

        # batched reciprocal of all 16 denominators, then normalize in place
        rd16 = consts.tile([16, TOK], bf16)
        with nc.allow_low_precision(reason="softmax denom recip in bf16"):
            nc.vector.reciprocal(rd16[:], dn16[:])
        for h in range(H):
            hp, hh = h // 2, h % 2
            psb = psum.tile([128, TOK], f32, tag="av")
            nc.tensor.matmul(psb[0:64, :],
                             lhsT=onehot[:, h * 64:(h + 1) * 64],
                             rhs=rd16[:], start=True, stop=True)
            nc.vector.tensor_tensor(
                out=attnT[hp][hh * 64:(hh + 1) * 64, :],
                in0=attnT[hp][hh * 64:(hh + 1) * 64, :],
                in1=psb[0:64, :], op=OP.mult)

        # evacuate the interleaved mod-rest accumulation
        nc.vector.tensor_tensor(out=mod_sb[:, 16:48], in0=ps_mod2[:],
                                in1=bias_sb[:, 16:48], op=OP.add)
        nc.vector.tensor_scalar_add(mod1p_sb[:, 16:48], mod_sb[:, 16:48], 1.0)
        if DEBUG:
            dump(dbg_mod[:], mod_sb)
            for _hp in range(KT):
                dump(dbg_attnT[_hp * 128:(_hp + 1) * 128, :], attnT[_hp])

        # ---------- Wo + residual1 (feature-major) ----------
        x2T = []
        for dc in range(KT):
            wblk = wpool.tile([128, D], bf16, tag="wblk", bufs=3)
            nc.sync.dma_start(wblk[:], wo_t[dc])
            ps_y = psum.tile([128, TOK], f32, tag="proj")
            for k in range(KT):
                nc.tensor.matmul(ps_y[:], lhsT=wblk[:, k * 128:(k + 1) * 128],
                                 rhs=attnT[k][:],
                                 start=(k == 0), stop=(k == KT - 1))
            ysc = trans.tile([128, TOK], f32, tag="sc_evac", name="ysc")
            nc.vector.tensor_scalar(
                out=ysc[:], in0=ps_y[:], scalar1=bcol(64 + dc),
                scalar2=mod_sb[:, 16 + dc:17 + dc], op0=OP.add, op1=OP.mult)
            x2t = persist.tile([128, TOK], f32, tag=f"x2T{dc}", name=f"x2T{dc}")
            nc.vector.tensor_tensor(out=x2t[:], in0=ysc[:],
                                    in1=xfeat_sb[dc][:], op=OP.add)
            x2T.append(x2t)
            if DEBUG:
                dump(dbg_x2T[dc * 128:(dc + 1) * 128, :], x2t)

        # ---------- LN2 + modulate ----------
        layer_norm_mod_feat(x2T, 32, 24, "h2")
        h2T = [hT_tiles[("h2", d)] for d in range(KT)]

        # ---------- MLP (token-local, two token-halves) ----------
        for half in range(2):
            tsl = slice(half * HT, (half + 1) * HT)
            G_sb = []
            for g4 in range(8):  # groups of 4 HID blocks
                w1q = wpool.tile([128, 8 * 512], bf16, tag="w1q", bufs=2)
                nc.sync.dma_start(w1q[:], w1_t[g4])
                ps_g = psum.tile([128, 4 * HT], f32, tag="big")
                for j in range(4):
                    hc = 4 * g4 + j
                    for k in range(KT):
                        nc.tensor.matmul(
                            ps_g[:, j * HT:(j + 1) * HT],
                            lhsT=w1q[:, k * 512 + j * 128:k * 512 + (j + 1) * 128],
                            rhs=h2T[k][:, tsl],
                            start=(k == 0 and j % 2 == 0),
                            stop=False, skip_group_check=True)
                    nc.tensor.matmul(
                        ps_g[:, j * HT:(j + 1) * HT],
                        lhsT=b1r_sb[:, hc * 128:(hc + 1) * 128],
                        rhs=ones_tok[:], start=False,
                        stop=(j % 2 == 1), skip_group_check=True)
                gt = persist.tile([128, 4 * HT], bf16, tag=f"G{g4}",
                                  name=f"G{g4}", bufs=1)
                nc.scalar.activation(out=gt[:], in_=ps_g[:], func=AF.Gelu)
                G_sb.append(gt)

            for dc in range(KT):
                ps_z = psum.tile([128, HT], f32, tag="proj")
                for kg2 in range(2):  # two [128, 2048] weight slabs
                    wblk = wpool.tile([128, 2048], bf16, tag="w2blk", bufs=2)
                    nc.sync.dma_start(wblk[:], w2_t[dc, kg2])
                    for i in range(16):
                        kb = 16 * kg2 + i
                        nc.tensor.matmul(
                            ps_z[:], lhsT=wblk[:, i * 128:(i + 1) * 128],
                            rhs=G_sb[kb // 4][:, (kb % 4) * HT:(kb % 4 + 1) * HT],
                            start=(kb == 0), stop=(kb == HC - 1))
                zsc = trans.tile([128, HT], f32, tag="sc_evac", name="zsc")
                nc.vector.tensor_scalar(
                    out=zsc[:], in0=ps_z[:], scalar1=bcol(72 + dc),
                    scalar2=mod_sb[:, 40 + dc:41 + dc],
                    op0=OP.add, op1=OP.mult)
                ot = trans.tile([128, HT], f32, tag="sc_evac", name="ot")
                nc.vector.tensor_tensor(out=ot[:], in0=zsc[:],
                                        in1=x2T[dc][:, tsl], op=OP.add)
                nc.gpsimd.dma_start(out_feat[dc * 128:(dc + 1) * 128, tsl],
                                    ot[:])

        ctx.close()

    nc.compile()
    return nc


def _pack_bias(bq, bk, bo, b2, bada):
    t = np.zeros((128, 80), np.float32)
    t[:, 0:48] = bada.reshape(48, 128).T
    t[:, 48:56] = bq.reshape(8, 128).T
    t[:, 56:64] = bk.reshape(8, 128).T
    t[:, 64:72] = bo.reshape(8, 128).T
    t[:, 72:80] = b2.reshape(8, 128).T
    return t


def _slab_oc(wT):
    """[D, D] W.T -> [8, 128, 1024]: slab[oc][p][k*128+c] = wT[k*128+p, oc*128+c]"""
    w = wT.reshape(KT, 128, KT, 128)          # [k, p, oc, c]
    return np.ascontiguousarray(w.transpose(2, 1, 0, 3).reshape(KT, 128, D))


def _slab_w1(w1T):
    """[D, HID] W1.T -> [8(g4), 128(p), 8(k)*512]: slab[g4][p][k*512+c] =
    w1T[k*128+p, g4*512+c]"""
    w = w1T.reshape(KT, 128, 8, 512)          # [k, p, g4, c]
    return np.ascontiguousarray(w.transpose(2, 1, 0, 3).reshape(KT, 128, 8 * 512))


def _slab_w2(w2T):
    """[HID, D] W2.T -> [8(dc), 2(kg2), 128(p), 16(i)*128]: slab[dc,kg2,p,i*128+c]
    = w2T[(16*kg2+i)*128+p, dc*128+c]"""
    w = w2T.reshape(2, 16, 128, KT, 128)      # [kg2, i, p, dc, c]
    return np.ascontiguousarray(
        w.transpose(3, 0, 2, 1, 4).reshape(KT, 2, 128, 2048))


def kernel(x, cond, Wq, bq, Wk, bk, Wv, bv, Wo, bo, W1, b1, W2, b2, Wada, bada):
    import ml_dtypes
    from concourse.bass_utils import run_bass_kernel_spmd

    bf = ml_dtypes.bfloat16
    if "nc" not in _cached:
        _cached["nc"] = _build()
    nc = _cached["nc"]

    x = np.asarray(x, np.float32)
    cond = np.asarray(cond, np.float32)
    to_bf_T = lambda w: np.ascontiguousarray(
        np.asarray(w, np.float32).T).astype(bf)
    wq_t = _slab_oc(np.asarray(Wq, np.float32).T).astype(bf)
    wk_t = _slab_oc(np.asarray(Wk, np.float32).T).astype(bf)
    wo_t = _slab_oc(np.asarray(Wo, np.float32).T).astype(bf)
    wvT = to_bf_T(Wv)
    w1_t = _slab_w1(np.asarray(W1, np.float32).T).astype(bf)
    w2_t = _slab_w2(np.asarray(W2, np.float32).T).astype(bf)
    wadaT = to_bf_T(Wada)
    biasc = _pack_bias(np.asarray(bq, np.float32), np.asarray(bk, np.float32),
                       np.asarray(bo, np.float32), np.asarray(b2, np.float32),
                       np.asarray(bada, np.float32))
    bv_row = np.asarray(bv, np.float32).reshape(1, D).astype(bf)
    onehot = np.repeat(np.eye(16, dtype=np.float32), 64, axis=1).astype(bf)

    in_maps = []
    for c in range(N_CORES):
        b, h = c // 2, c % 2
        xs = x[b, h * TOK:(h + 1) * TOK, :]
        in_maps.append({
            "x_feat": np.ascontiguousarray(xs.T),
            "condT": np.ascontiguousarray(cond[b, 0].reshape(8, 128).T),
            "wq_t": wq_t, "wk_t": wk_t, "wo_t": wo_t, "wvT": wvT,
            "w1_t": w1_t, "w2_t": w2_t, "wadaT": wadaT,
            "biasc": biasc, "bv_row": bv_row,
            "b1_row": np.asarray(b1, np.float32).reshape(1, HID).astype(bf),
            "onehot_d": onehot,
        })

    _cached["in_maps"] = in_maps
    res = run_bass_kernel_spmd(nc, in_maps, core_ids=list(range(N_CORES)))
    _cached["results"] = res.results
    out = np.empty((B, S, D), np.float32)
    for c in range(N_CORES):
        b, h = c // 2, c % 2
        out[b, h * TOK:(h + 1) * TOK, :] = res.results[c]["out_feat"].T
    return out
